# revision 5
# baseline (speedup 1.0000x reference)
"""Trainium2 Bass kernel for nn_ComposerLayer (sparse_attention).

Math (reference, per batch b):
    OQ[o,h]   = sum_s operators[s,o] * Wq_op[h,s] + bq_op[h]          # [O,H] (output 2)
    keys      = x @ Wk.T + bk
    logits    = keys @ OQ.T / sqrt(H)                                  # [S,O]
    RW        = softmax_s(logits); OW = softmax_o(logits)
    OOut[o,h] = sum_s (x @ Wv.T + bv)[s,h] * RW[s,o] * operators[s,o]
    out[s,h]  = sum_o OW[s,o] * OOut[o,h]                              # (output 1)

Folded form used here (exact up to fp reassociation):
    KQ[h',o] = sum_h Wk[h,h'] * OQ[o,h]       ; c[o] = sum_h bk[h]*OQ[o,h]/sqrt(H)
    logits   = x @ KQ / sqrt(H) + c           # keys never materialized
    P        = exp(logits)                    # logits are O(1), no max-sub needed
    Z[o]     = sum_s P ; sumo[s] = sum_o P
    G        = P * operators                  # [S,O]
    Ag[o,h'] = sum_s G[s,o] * x[s,h'] ; sG[o] = sum_s G[s,o]
    OOut     = (Ag @ Wv.T + sG*bv) / Z        # values never materialized
    out      = (P.T/sumo preserved per-s) : out[s,h] = sum_o P[s,o]*OOut[o,h]/sumo[s]

Sharding: data-parallel over batch B=16 across 8 cores (2 batches/core);
all weights replicated.  All heavy matmuls run with float32r (full-rate fp32
on the PE for moving dim >= 256).
"""

import math
import threading

import numpy as np

import concourse.bass as bass  # noqa: F401  (bass types referenced via APs)
import concourse.mybir as mybir
import concourse.tile as tile
from concourse import bacc
from concourse.bass_utils import run_bass_kernel_spmd
from concourse.masks import make_identity

B, S, H, O = 16, 2048, 512, 16
NCORES = 8
BL = B // NCORES          # batches per core
HC = H // 128             # 4 chunks of h (and h')
SC = S // 128             # 16 chunks of s
F32 = mybir.dt.float32
F32R = mybir.dt.float32r
Exp = mybir.ActivationFunctionType.Exp
Copy = mybir.ActivationFunctionType.Copy
AX = mybir.AxisListType.X
INV_SQRT_H = 1.0 / math.sqrt(H)


def _r(ap):
    """Bitcast an fp32 AP to float32r for full-rate PE matmul."""
    return ap.bitcast(F32R)


def build_nc():
    nc = bacc.Bacc("TRN2", target_bir_lowering=False, debug=False,
                   enable_asserts=False)

    xs = nc.dram_tensor("x", [BL, S, H], F32, kind="ExternalInput")
    Wv = nc.dram_tensor("Wv", [H, H], F32, kind="ExternalInput")
    bv = nc.dram_tensor("bv", [H], F32, kind="ExternalInput")
    Wk = nc.dram_tensor("Wk", [H, H], F32, kind="ExternalInput")
    bk = nc.dram_tensor("bk", [H], F32, kind="ExternalInput")
    Wq = nc.dram_tensor("Wq_op", [H, S], F32, kind="ExternalInput")
    bq = nc.dram_tensor("bq_op", [H], F32, kind="ExternalInput")
    ops = nc.dram_tensor("operators", [S, O], F32, kind="ExternalInput")
    out = nc.dram_tensor("out", [BL, S, H], F32, kind="ExternalOutput")
    oq = nc.dram_tensor("oq", [O, H], F32, kind="ExternalOutput")

    with tile.TileContext(nc) as tc:
        with (
            tc.tile_pool(name="persist", bufs=1) as persist,
            tc.tile_pool(name="xp", bufs=2) as xp,
            tc.tile_pool(name="big_ps", bufs=3, space="PSUM") as big_ps,
            tc.tile_pool(name="lg_ps", bufs=2, space="PSUM") as lg_ps,
            tc.tile_pool(name="sm_ps", bufs=3, space="PSUM") as sm_ps,
        ):
            # ---- early: start x loads (critical path) ----
            x_sb = []
            for b in range(BL):
                xt_ = xp.tile([128, SC, H], F32R, tag="x")
                xr = xs.ap()[b].rearrange("(c p) h -> p c h", p=128).bitcast(F32R)
                for g in range(4):
                    nc.sync.dma_start(out=xt_[:, g * 4:(g + 1) * 4, :],
                                      in_=xr[:, g * 4:(g + 1) * 4, :])
                x_sb.append(xt_)

            # ---- persistent small constants ----
            I128 = persist.tile([128, 128], F32)
            make_identity(nc, I128[:])
            ones_col = persist.tile([128, 1], F32)
            nc.vector.memset(ones_col[:], 1.0)
            opSO = persist.tile([128, SC, O], F32R)   # operators [s,o]
            nc.sync.dma_start(out=opSO[:],
                              in_=ops.ap().rearrange("(c p) o -> p c o", p=128).bitcast(F32R))
            bk_sb = persist.tile([128, HC], F32R)
            nc.sync.dma_start(out=bk_sb[:],
                              in_=bk.ap().rearrange("(c p) -> p c", p=128).bitcast(F32R))
            bv_row = persist.tile([1, H], F32R)
            nc.sync.dma_start(out=bv_row[:], in_=bv.ap().unsqueeze(0).bitcast(F32R))
            bq_bc = persist.tile([O, H], F32)
            nc.gpsimd.dma_start(out=bq_bc[:],
                                in_=bq.ap().partition_broadcast(O))
            WvT = persist.tile([128, HC, H], F32R)    # Wv.T : [h' , h]
            KQ = persist.tile([128, HC, O], F32R)     # [h', o]
            c_sb = persist.tile([O, 1], F32)
            OQ_sb = persist.tile([O, H], F32)

            # ---- phase 0: OQ, KQ, c, WvT (scoped weights) ----
            with tc.tile_pool(name="ph0", bufs=1) as ph0:
                Wq_sb = ph0.tile([128, HC, S], F32)      # Wq_op [h, s]
                nc.sync.dma_start(
                    out=Wq_sb[:],
                    in_=Wq.ap().rearrange("(c p) s -> p c s", p=128))
                WqT = ph0.tile([128, SC, H], F32R)        # Wq_op.T [s, h]
                Wk_sb = ph0.tile([128, HC, H], F32R)      # Wk [h, h']
                nc.sync.dma_start(
                    out=Wk_sb[:],
                    in_=Wk.ap().rearrange("(c p) h2 -> p c h2", p=128).bitcast(F32R))
                Wv_sb = ph0.tile([128, HC, H], F32)      # Wv [h, h']
                nc.sync.dma_start(
                    out=Wv_sb[:],
                    in_=Wv.ap().rearrange("(c p) h2 -> p c h2", p=128))

                # transpose Wq_op -> WqT  (64 PE transposes, 16 copies)
                for sc in range(SC):
                    pt = big_ps.tile([128, H], F32, tag="big")
                    for hc in range(HC):
                        nc.tensor.transpose(
                            pt[:, hc * 128:(hc + 1) * 128],
                            Wq_sb[:, hc, sc * 128:(sc + 1) * 128], I128[:])
                    if sc % 2 == 0:
                        nc.vector.tensor_copy(WqT[:, sc, :], pt[:])
                    else:
                        nc.scalar.copy(WqT[:, sc, :], pt[:])

                # transpose Wv -> WvT
                for hpc in range(HC):
                    pt = big_ps.tile([128, H], F32, tag="big")
                    for hc in range(HC):
                        nc.tensor.transpose(
                            pt[:, hc * 128:(hc + 1) * 128],
                            Wv_sb[:, hc, hpc * 128:(hpc + 1) * 128], I128[:])
                    if hpc % 2 == 0:
                        nc.vector.tensor_copy(WvT[:, hpc, :], pt[:])
                    else:
                        nc.scalar.copy(WvT[:, hpc, :], pt[:])

                # OQ = operators.T @ Wq_op.T  (accumulate over s chunks)
                oq_ps = lg_ps.tile([O, H], F32, tag="lg")
                for sc in range(SC):
                    nc.tensor.matmul(oq_ps[:], _r(opSO[:, sc, :]),
                                     _r(WqT[:, sc, :]),
                                     start=(sc == 0), stop=(sc == SC - 1))
                nc.vector.tensor_add(OQ_sb[:], oq_ps[:], bq_bc[:])
                nc.sync.dma_start(out=oq.ap(), in_=OQ_sb[:])

                # OQT [h, o]
                oqt_ps = sm_ps.tile([128, HC, O], F32, tag="sm")
                for hc in range(HC):
                    nc.tensor.transpose(
                        oqt_ps[:, hc, :],
                        OQ_sb[:, hc * 128:(hc + 1) * 128], I128[:16, :16])
                OQT = persist.tile([128, HC, O], F32R)
                nc.vector.tensor_copy(OQT[:], oqt_ps[:])

                # KQ[h',o] = sum_h Wk[h,h'] OQ[o,h]
                kq_ps = sm_ps.tile([128, HC, O], F32, tag="sm")
                for mc in range(HC):
                    for kc in range(HC):
                        nc.tensor.matmul(
                            kq_ps[:, mc, :],
                            Wk_sb[:, kc, mc * 128:(mc + 1) * 128].bitcast(F32),
                            OQT[:, kc, :].bitcast(F32),
                            start=(kc == 0), stop=(kc == HC - 1))
                nc.vector.tensor_copy(KQ[:], kq_ps[:])

                # c[o] = sum_h bk[h] OQ[o,h] / sqrt(H)
                c_ps = sm_ps.tile([O, 1], F32, tag="sm")
                for kc in range(HC):
                    nc.tensor.matmul(c_ps[:],
                                     OQT[:, kc, :].bitcast(F32),
                                     bk_sb[:, kc:kc + 1].bitcast(F32),
                                     start=(kc == 0), stop=(kc == HC - 1))
                nc.vector.tensor_scalar_mul(c_sb[:], c_ps[:], INV_SQRT_H)

            # ---- per-batch pipeline ----
            with (
                tc.tile_pool(name="xtp", bufs=2) as xtp,
                tc.tile_pool(name="ptp", bufs=2) as ptp,
                tc.tile_pool(name="smp", bufs=2) as smp,
                tc.tile_pool(name="outp", bufs=4) as outp,
            ):
                for b in range(BL):
                    xT = xtp.tile([128, HC, S], F32R, tag="xT")  # x.T [h', s]
                    # transpose x -> xT
                    for g in range(4):
                        for hc in range(HC):
                            pt = big_ps.tile([128, 512], F32, tag="big")
                            for k in range(4):
                                sc = g * 4 + k
                                nc.tensor.transpose(
                                    pt[:, k * 128:(k + 1) * 128],
                                    x_sb[b][:, sc, hc * 128:(hc + 1) * 128]
                                    .bitcast(F32), I128[:])
                            if (g + hc) % 2 == 0:
                                nc.vector.tensor_copy(
                                    xT[:, hc, g * 512:(g + 1) * 512], pt[:])
                            else:
                                nc.scalar.copy(
                                    xT[:, hc, g * 512:(g + 1) * 512], pt[:])

                    # logits.T [o, s] by s-groups; exp with accumulated Z
                    PT = ptp.tile([O, S], F32R, tag="PT")       # exp(logits).T
                    Zp = smp.tile([O, 4], F32, tag="Zp")
                    for sg in range(4):
                        lg = lg_ps.tile([O, 512], F32, tag="lg")
                        for hc in range(HC):
                            nc.tensor.matmul(
                                lg[:], _r(KQ[:, hc, :]),
                                _r(xT[:, hc, sg * 512:(sg + 1) * 512]),
                                start=(hc == 0), stop=(hc == HC - 1))
                        nc.scalar.activation(
                            out=PT[:, sg * 512:(sg + 1) * 512], in_=lg[:],
                            func=Exp, bias=c_sb[:], scale=INV_SQRT_H,
                            accum_out=Zp[:, sg:sg + 1])
                    Z = smp.tile([O, 1], F32, tag="Z")
                    nc.vector.reduce_sum(Z[:], Zp[:], axis=AX)
                    rZ = smp.tile([O, 1], F32, tag="rZ")
                    nc.vector.reciprocal(rZ[:], Z[:])

                    # P [s, o] via PE transpose of PT
                    P = smp.tile([128, SC, O], F32, tag="P")
                    for g in range(4):
                        pp = sm_ps.tile([128, 4, O], F32, tag="sm")
                        for k in range(4):
                            sc = g * 4 + k
                            nc.tensor.transpose(
                                pp[:, k, :],
                                PT[:, sc * 128:(sc + 1) * 128].bitcast(F32),
                                I128[:16, :16])
                        nc.vector.tensor_copy(P[:, g * 4:(g + 1) * 4, :], pp[:])

                    # sumo[s] = sum_o P ; rsumo = 1/sumo
                    sumo = smp.tile([128, SC], F32, tag="sumo")
                    nc.vector.reduce_sum(sumo[:], P[:], axis=AX)
                    rsumo = smp.tile([128, SC], F32, tag="rsumo")
                    nc.vector.reciprocal(rsumo[:], sumo[:])

                    # G = P * operators
                    G = smp.tile([128, SC, O], F32R, tag="G")
                    nc.vector.tensor_mul(G[:], P[:], opSO[:].bitcast(F32))

                    # Ag[o,h'] = sum_s G[s,o] x[s,h'] ; sG[o] = sum_s G[s,o]
                    ag_ps = lg_ps.tile([O, H], F32, tag="lg")
                    for sc in range(SC):
                        nc.tensor.matmul(ag_ps[:], _r(G[:, sc, :]),
                                         _r(x_sb[b][:, sc, :]),
                                         start=(sc == 0), stop=(sc == SC - 1))
                    sg_ps = sm_ps.tile([1, O], F32, tag="sm")
                    for sc in range(SC):
                        nc.tensor.matmul(sg_ps[:], ones_col[:],
                                         G[:, sc, :].bitcast(F32),
                                         start=(sc == 0), stop=(sc == SC - 1))
                    Ag = smp.tile([O, H], F32, tag="Ag")
                    nc.vector.tensor_copy(Ag[:], ag_ps[:])
                    sG = smp.tile([1, O], F32R, tag="sG")
                    nc.vector.tensor_copy(sG[:], sg_ps[:])

                    # AgT [h', o]
                    agt_ps = sm_ps.tile([128, HC, O], F32, tag="sm")
                    for hc in range(HC):
                        nc.tensor.transpose(
                            agt_ps[:, hc, :],
                            Ag[:, hc * 128:(hc + 1) * 128], I128[:16, :16])
                    AgT = smp.tile([128, HC, O], F32R, tag="AgT")
                    nc.vector.tensor_copy(AgT[:], agt_ps[:])

                    # OOut = (Ag @ Wv.T + sG*bv) / Z
                    oo_ps = lg_ps.tile([O, H], F32, tag="lg")
                    for hc in range(HC):
                        nc.tensor.matmul(oo_ps[:], _r(AgT[:, hc, :]),
                                         _r(WvT[:, hc, :]),
                                         start=(hc == 0), stop=False)
                    nc.tensor.matmul(oo_ps[:], _r(sG[:]), _r(bv_row[:]),
                                     start=False, stop=True)
                    OO = smp.tile([O, H], F32R, tag="OO")
                    nc.vector.tensor_scalar_mul(OO[:], oo_ps[:], rZ[:])

                    # out[s,h] = sum_o PT[o,s] OO[o,h] / sumo[s]
                    outr = out.ap()[b].rearrange("(c p) h -> p c h", p=128)
                    for sc in range(SC):
                        op_ = big_ps.tile([128, H], F32, tag="big")
                        nc.tensor.matmul(
                            op_[:], _r(PT[:, sc * 128:(sc + 1) * 128]),
                            _r(OO[:]), start=True, stop=True)
                        ot = outp.tile([128, H], F32, tag="ot")
                        if sc % 2 == 0:
                            nc.vector.tensor_scalar_mul(
                                ot[:], op_[:], rsumo[:, sc:sc + 1])
                        else:
                            nc.scalar.activation(
                                out=ot[:], in_=op_[:], func=Copy,
                                bias=0.0, scale=rsumo[:, sc:sc + 1])
                        nc.sync.dma_start(out=outr[:, sc, :], in_=ot[:])

    nc.finalize()
    return nc


_lock = threading.Lock()
_nc_cache = []


def _get_nc():
    with _lock:
        if not _nc_cache:
            _nc_cache.append(build_nc())
        return _nc_cache[0]


def kernel(x, Wv, bv, Wk, bk, Wq_op, bq_op, operators):
    x = np.ascontiguousarray(np.asarray(x, dtype=np.float32))
    shared = {
        "Wv": np.ascontiguousarray(np.asarray(Wv, np.float32)),
        "bv": np.ascontiguousarray(np.asarray(bv, np.float32)),
        "Wk": np.ascontiguousarray(np.asarray(Wk, np.float32)),
        "bk": np.ascontiguousarray(np.asarray(bk, np.float32)),
        "Wq_op": np.ascontiguousarray(np.asarray(Wq_op, np.float32)),
        "bq_op": np.ascontiguousarray(np.asarray(bq_op, np.float32)),
        "operators": np.ascontiguousarray(np.asarray(operators, np.float32)),
    }
    nc = _get_nc()
    in_maps = [
        {"x": np.ascontiguousarray(x[c * BL:(c + 1) * BL]), **shared}
        for c in range(NCORES)
    ]
    res = run_bass_kernel_spmd(nc, in_maps, core_ids=list(range(NCORES)))
    output = np.concatenate([res.results[c]["out"] for c in range(NCORES)],
                            axis=0)
    operator_queries = res.results[0]["oq"]
    return output, operator_queries


# revision 6
# speedup vs baseline: 5.6691x; 5.6691x over previous
"""Trainium2 Bass kernel for nn_ComposerLayer (sparse_attention).

Math (reference, per batch b):
    OQ[o,h]   = sum_s operators[s,o] * Wq_op[h,s] + bq_op[h]          # [O,H] (output 2)
    keys      = x @ Wk.T + bk
    logits    = keys @ OQ.T / sqrt(H)                                  # [S,O]
    RW        = softmax_s(logits); OW = softmax_o(logits)
    OOut[o,h] = sum_s (x @ Wv.T + bv)[s,h] * RW[s,o] * operators[s,o]
    out[s,h]  = sum_o OW[s,o] * OOut[o,h]                              # (output 1)

Folded form used here (exact up to fp reassociation):
    KQ[h',o] = sum_h Wk[h,h'] * OQ[o,h]       ; c[o] = sum_h bk[h]*OQ[o,h]/sqrt(H)
    logits   = x @ KQ / sqrt(H) + c           # keys never materialized
    P        = exp(logits)                    # logits are O(1), no max-sub needed
    Z[o]     = sum_s P ; sumo[s] = sum_o P
    G        = P * operators                  # [S,O]
    Ag[o,h'] = sum_s G[s,o] * x[s,h'] ; sG[o] = sum_s G[s,o]
    OOut     = (Ag @ Wv.T + sG*bv) / Z        # values never materialized
    out      = (P.T/sumo preserved per-s) : out[s,h] = sum_o P[s,o]*OOut[o,h]/sumo[s]

Sharding: data-parallel over batch B=16 across 8 cores (2 batches/core);
all weights replicated.  All heavy matmuls run with float32r (full-rate fp32
on the PE for moving dim >= 256).
"""

import math
import threading

import numpy as np

import concourse.bass as bass  # noqa: F401  (bass types referenced via APs)
import concourse.mybir as mybir
import concourse.tile as tile
from concourse import bacc
from concourse.bass_utils import run_bass_kernel_spmd
from concourse.masks import make_identity

B, S, H, O = 16, 2048, 512, 16
NCORES = 8
BL = B // NCORES          # batches per core
HC = H // 128             # 4 chunks of h (and h')
SC = S // 128             # 16 chunks of s
F32 = mybir.dt.float32
F32R = mybir.dt.float32r
Exp = mybir.ActivationFunctionType.Exp
Copy = mybir.ActivationFunctionType.Copy
AX = mybir.AxisListType.X
INV_SQRT_H = 1.0 / math.sqrt(H)


def _r(ap):
    """Bitcast an fp32 AP to float32r for full-rate PE matmul."""
    return ap.bitcast(F32R)


def build_nc(repeat: int = 1):
    nc = bacc.Bacc("TRN2", target_bir_lowering=False, debug=False,
                   enable_asserts=False)

    xs = nc.dram_tensor("x", [BL, S, H], F32, kind="ExternalInput")
    Wv = nc.dram_tensor("Wv", [H, H], F32, kind="ExternalInput")
    bv = nc.dram_tensor("bv", [H], F32, kind="ExternalInput")
    Wk = nc.dram_tensor("Wk", [H, H], F32, kind="ExternalInput")
    bk = nc.dram_tensor("bk", [H], F32, kind="ExternalInput")
    Wq = nc.dram_tensor("Wq_op", [H, S], F32, kind="ExternalInput")
    bq = nc.dram_tensor("bq_op", [H], F32, kind="ExternalInput")
    ops = nc.dram_tensor("operators", [S, O], F32, kind="ExternalInput")
    out = nc.dram_tensor("out", [BL, S, H], F32, kind="ExternalOutput")
    oq = nc.dram_tensor("oq", [O, H], F32, kind="ExternalOutput")

    with tile.TileContext(nc) as tc:
        with (
            tc.tile_pool(name="persist", bufs=1) as persist,
            tc.tile_pool(name="xp", bufs=2) as xp,
            tc.tile_pool(name="big_ps", bufs=3, space="PSUM") as big_ps,
            tc.tile_pool(name="lg_ps", bufs=2, space="PSUM") as lg_ps,
            tc.tile_pool(name="sm_ps", bufs=3, space="PSUM") as sm_ps,
        ):
            # ---- early: start x loads (critical path) ----
            x_sb = []
            for b in range(BL):
                xt_ = xp.tile([128, SC, H], F32R, tag="x")
                xr = xs.ap()[b].rearrange("(c p) h -> p c h", p=128).bitcast(F32R)
                for g in range(4):
                    nc.sync.dma_start(out=xt_[:, g * 4:(g + 1) * 4, :],
                                      in_=xr[:, g * 4:(g + 1) * 4, :])
                x_sb.append(xt_)

            # ---- persistent small constants ----
            I128 = persist.tile([128, 128], F32)
            make_identity(nc, I128[:])
            ones_col = persist.tile([128, 1], F32)
            nc.vector.memset(ones_col[:], 1.0)
            opSO = persist.tile([128, SC, O], F32R)   # operators [s,o]
            nc.sync.dma_start(out=opSO[:],
                              in_=ops.ap().rearrange("(c p) o -> p c o", p=128).bitcast(F32R))
            bk_sb = persist.tile([128, HC], F32R)
            nc.sync.dma_start(out=bk_sb[:],
                              in_=bk.ap().rearrange("(c p) -> p c", p=128).bitcast(F32R))
            bv_row = persist.tile([1, H], F32R)
            nc.sync.dma_start(out=bv_row[:], in_=bv.ap().unsqueeze(0).bitcast(F32R))
            bq_bc = persist.tile([O, H], F32)
            nc.gpsimd.dma_start(out=bq_bc[:],
                                in_=bq.ap().partition_broadcast(O))
            WvT = persist.tile([128, HC, H], F32R)    # Wv.T : [h' , h]
            KQ = persist.tile([128, HC, O], F32R)     # [h', o]
            c_sb = persist.tile([O, 1], F32)
            OQ_sb = persist.tile([O, H], F32)

            # ---- phase 0: OQ, KQ, c, WvT (scoped weights) ----
            with tc.tile_pool(name="ph0", bufs=1) as ph0:
                Wq_sb = ph0.tile([128, HC, S], F32)      # Wq_op [h, s]
                nc.sync.dma_start(
                    out=Wq_sb[:],
                    in_=Wq.ap().rearrange("(c p) s -> p c s", p=128))
                WqT = ph0.tile([128, SC, H], F32R)        # Wq_op.T [s, h]
                Wk_sb = ph0.tile([128, HC, H], F32R)      # Wk [h, h']
                nc.sync.dma_start(
                    out=Wk_sb[:],
                    in_=Wk.ap().rearrange("(c p) h2 -> p c h2", p=128).bitcast(F32R))
                Wv_sb = ph0.tile([128, HC, H], F32)      # Wv [h, h']
                nc.sync.dma_start(
                    out=Wv_sb[:],
                    in_=Wv.ap().rearrange("(c p) h2 -> p c h2", p=128))

                # transpose Wq_op -> WqT  (64 PE transposes, 16 copies)
                for sc in range(SC):
                    pt = big_ps.tile([128, H], F32, tag="big")
                    for hc in range(HC):
                        nc.tensor.transpose(
                            pt[:, hc * 128:(hc + 1) * 128],
                            Wq_sb[:, hc, sc * 128:(sc + 1) * 128], I128[:])
                    if sc % 2 == 0:
                        nc.vector.tensor_copy(WqT[:, sc, :], pt[:])
                    else:
                        nc.scalar.copy(WqT[:, sc, :], pt[:])

                # transpose Wv -> WvT
                for hpc in range(HC):
                    pt = big_ps.tile([128, H], F32, tag="big")
                    for hc in range(HC):
                        nc.tensor.transpose(
                            pt[:, hc * 128:(hc + 1) * 128],
                            Wv_sb[:, hc, hpc * 128:(hpc + 1) * 128], I128[:])
                    if hpc % 2 == 0:
                        nc.vector.tensor_copy(WvT[:, hpc, :], pt[:])
                    else:
                        nc.scalar.copy(WvT[:, hpc, :], pt[:])

                # OQ = operators.T @ Wq_op.T  (accumulate over s chunks)
                oq_ps = lg_ps.tile([O, H], F32, tag="lg")
                for sc in range(SC):
                    nc.tensor.matmul(oq_ps[:], _r(opSO[:, sc, :]),
                                     _r(WqT[:, sc, :]),
                                     start=(sc == 0), stop=(sc == SC - 1))
                nc.vector.tensor_add(OQ_sb[:], oq_ps[:], bq_bc[:])
                nc.sync.dma_start(out=oq.ap(), in_=OQ_sb[:])

                # OQT [h, o]
                oqt_ps = sm_ps.tile([128, HC, O], F32, tag="sm")
                for hc in range(HC):
                    nc.tensor.transpose(
                        oqt_ps[:, hc, :],
                        OQ_sb[:, hc * 128:(hc + 1) * 128], I128[:16, :16])
                OQT = persist.tile([128, HC, O], F32R)
                nc.vector.tensor_copy(OQT[:], oqt_ps[:])

                # KQ[h',o] = sum_h Wk[h,h'] OQ[o,h]
                kq_ps = sm_ps.tile([128, HC, O], F32, tag="sm")
                for mc in range(HC):
                    for kc in range(HC):
                        nc.tensor.matmul(
                            kq_ps[:, mc, :],
                            Wk_sb[:, kc, mc * 128:(mc + 1) * 128].bitcast(F32),
                            OQT[:, kc, :].bitcast(F32),
                            start=(kc == 0), stop=(kc == HC - 1))
                nc.vector.tensor_copy(KQ[:], kq_ps[:])

                # c[o] = sum_h bk[h] OQ[o,h] / sqrt(H)
                c_ps = sm_ps.tile([O, 1], F32, tag="sm")
                for kc in range(HC):
                    nc.tensor.matmul(c_ps[:],
                                     OQT[:, kc, :].bitcast(F32),
                                     bk_sb[:, kc:kc + 1].bitcast(F32),
                                     start=(kc == 0), stop=(kc == HC - 1))
                nc.vector.tensor_scalar_mul(c_sb[:], c_ps[:], INV_SQRT_H)

            # ---- per-batch pipeline ----
            with (
                tc.tile_pool(name="xtp", bufs=2) as xtp,
                tc.tile_pool(name="ptp", bufs=2) as ptp,
                tc.tile_pool(name="smp", bufs=2) as smp,
                tc.tile_pool(name="outp", bufs=4) as outp,
            ):
              for _rep in range(repeat):
                for b in range(BL):
                    xT = xtp.tile([128, HC, S], F32R, tag="xT")  # x.T [h', s]
                    # transpose x -> xT
                    for g in range(4):
                        for hc in range(HC):
                            pt = big_ps.tile([128, 512], F32, tag="big")
                            for k in range(4):
                                sc = g * 4 + k
                                nc.tensor.transpose(
                                    pt[:, k * 128:(k + 1) * 128],
                                    x_sb[b][:, sc, hc * 128:(hc + 1) * 128]
                                    .bitcast(F32), I128[:])
                            if (g + hc) % 2 == 0:
                                nc.vector.tensor_copy(
                                    xT[:, hc, g * 512:(g + 1) * 512], pt[:])
                            else:
                                nc.scalar.copy(
                                    xT[:, hc, g * 512:(g + 1) * 512], pt[:])

                    # logits.T [o, s] by s-groups; exp with accumulated Z
                    PT = ptp.tile([O, S], F32R, tag="PT")       # exp(logits).T
                    Zp = smp.tile([O, 4], F32, tag="Zp")
                    for sg in range(4):
                        lg = lg_ps.tile([O, 512], F32, tag="lg")
                        for hc in range(HC):
                            nc.tensor.matmul(
                                lg[:], _r(KQ[:, hc, :]),
                                _r(xT[:, hc, sg * 512:(sg + 1) * 512]),
                                start=(hc == 0), stop=(hc == HC - 1))
                        nc.scalar.activation(
                            out=PT[:, sg * 512:(sg + 1) * 512], in_=lg[:],
                            func=Exp, bias=c_sb[:], scale=INV_SQRT_H,
                            accum_out=Zp[:, sg:sg + 1])
                    Z = smp.tile([O, 1], F32, tag="Z")
                    nc.vector.reduce_sum(Z[:], Zp[:], axis=AX)
                    rZ = smp.tile([O, 1], F32, tag="rZ")
                    nc.vector.reciprocal(rZ[:], Z[:])

                    # P [s, o] via PE transpose of PT
                    P = smp.tile([128, SC, O], F32, tag="P")
                    for g in range(4):
                        pp = sm_ps.tile([128, 4, O], F32, tag="sm")
                        for k in range(4):
                            sc = g * 4 + k
                            nc.tensor.transpose(
                                pp[:, k, :],
                                PT[:, sc * 128:(sc + 1) * 128].bitcast(F32),
                                I128[:16, :16])
                        nc.vector.tensor_copy(P[:, g * 4:(g + 1) * 4, :], pp[:])

                    # sumo[s] = sum_o P ; rsumo = 1/sumo
                    sumo = smp.tile([128, SC], F32, tag="sumo")
                    nc.vector.reduce_sum(sumo[:], P[:], axis=AX)
                    rsumo = smp.tile([128, SC], F32, tag="rsumo")
                    nc.vector.reciprocal(rsumo[:], sumo[:])

                    # G = P * operators
                    G = smp.tile([128, SC, O], F32R, tag="G")
                    nc.vector.tensor_mul(G[:], P[:], opSO[:].bitcast(F32))

                    # Ag[o,h'] = sum_s G[s,o] x[s,h'] ; sG[o] = sum_s G[s,o]
                    ag_ps = lg_ps.tile([O, H], F32, tag="lg")
                    for sc in range(SC):
                        nc.tensor.matmul(ag_ps[:], _r(G[:, sc, :]),
                                         _r(x_sb[b][:, sc, :]),
                                         start=(sc == 0), stop=(sc == SC - 1))
                    sg_ps = sm_ps.tile([1, O], F32, tag="sm")
                    for sc in range(SC):
                        nc.tensor.matmul(sg_ps[:], ones_col[:],
                                         G[:, sc, :].bitcast(F32),
                                         start=(sc == 0), stop=(sc == SC - 1))
                    Ag = smp.tile([O, H], F32, tag="Ag")
                    nc.vector.tensor_copy(Ag[:], ag_ps[:])
                    sG = smp.tile([1, O], F32R, tag="sG")
                    nc.vector.tensor_copy(sG[:], sg_ps[:])

                    # AgT [h', o]
                    agt_ps = sm_ps.tile([128, HC, O], F32, tag="sm")
                    for hc in range(HC):
                        nc.tensor.transpose(
                            agt_ps[:, hc, :],
                            Ag[:, hc * 128:(hc + 1) * 128], I128[:16, :16])
                    AgT = smp.tile([128, HC, O], F32R, tag="AgT")
                    nc.vector.tensor_copy(AgT[:], agt_ps[:])

                    # OOut = (Ag @ Wv.T + sG*bv) / Z
                    oo_ps = lg_ps.tile([O, H], F32, tag="lg")
                    for hc in range(HC):
                        nc.tensor.matmul(oo_ps[:], _r(AgT[:, hc, :]),
                                         _r(WvT[:, hc, :]),
                                         start=(hc == 0), stop=False)
                    nc.tensor.matmul(oo_ps[:], _r(sG[:]), _r(bv_row[:]),
                                     start=False, stop=True)
                    OO = smp.tile([O, H], F32R, tag="OO")
                    nc.vector.tensor_scalar_mul(OO[:], oo_ps[:], rZ[:])

                    # out[s,h] = sum_o PT[o,s] OO[o,h] / sumo[s]
                    outr = out.ap()[b].rearrange("(c p) h -> p c h", p=128)
                    for sc in range(SC):
                        op_ = big_ps.tile([128, H], F32, tag="big")
                        nc.tensor.matmul(
                            op_[:], _r(PT[:, sc * 128:(sc + 1) * 128]),
                            _r(OO[:]), start=True, stop=True)
                        ot = outp.tile([128, H], F32, tag="ot")
                        if sc % 2 == 0:
                            nc.vector.tensor_scalar_mul(
                                ot[:], op_[:], rsumo[:, sc:sc + 1])
                        else:
                            nc.scalar.activation(
                                out=ot[:], in_=op_[:], func=Copy,
                                bias=0.0, scale=rsumo[:, sc:sc + 1])
                        nc.sync.dma_start(out=outr[:, sc, :], in_=ot[:])

    nc.finalize()
    return nc


_lock = threading.Lock()
_nc_cache = []


def _get_nc():
    with _lock:
        if not _nc_cache:
            _nc_cache.append(build_nc())
        return _nc_cache[0]


def kernel(x, Wv, bv, Wk, bk, Wq_op, bq_op, operators):
    x = np.ascontiguousarray(np.asarray(x, dtype=np.float32))
    shared = {
        "Wv": np.ascontiguousarray(np.asarray(Wv, np.float32)),
        "bv": np.ascontiguousarray(np.asarray(bv, np.float32)),
        "Wk": np.ascontiguousarray(np.asarray(Wk, np.float32)),
        "bk": np.ascontiguousarray(np.asarray(bk, np.float32)),
        "Wq_op": np.ascontiguousarray(np.asarray(Wq_op, np.float32)),
        "bq_op": np.ascontiguousarray(np.asarray(bq_op, np.float32)),
        "operators": np.ascontiguousarray(np.asarray(operators, np.float32)),
    }
    nc = _get_nc()
    in_maps = [
        {"x": np.ascontiguousarray(x[c * BL:(c + 1) * BL]), **shared}
        for c in range(NCORES)
    ]
    res = run_bass_kernel_spmd(nc, in_maps, core_ids=list(range(NCORES)))
    output = np.concatenate([res.results[c]["out"] for c in range(NCORES)],
                            axis=0)
    operator_queries = res.results[0]["oq"]
    return output, operator_queries
